# revision 11
# baseline (speedup 1.0000x reference)
"""Conv2dfft kernel for Trainium2 (8 NeuronCores, SPMD data-parallel over N).

The reference computes an FFT-based 2D cross-correlation that is exactly a
3x3 same-padding conv2d: out[n,f,h,w] = sum_{c,ky,kx} x[n,c,h+ky-1,w+kx-1]
* weight[f,c,ky,kx] + bias[f]  (zero-padded at the borders).

We implement it directly as 9 shifted 128x128 matmuls accumulated in PSUM:
the contraction dim C=128 fills the PE partition dim, F=128 fills the output
partition dim. Data-parallel: 32 images / 8 cores = 4 images per core.

Trace-driven layout:
- fp32r and bf16 stream the PE at the same 1 row/cycle steady rate, so the
  matmul dtype does not change compute time -- but bf16 halves the input DMA
  bytes, which is what gates the FIRST matmul. x and w are shipped as bf16.
- The weight tensor is split in two and issued on BOTH hardware DGE queues
  (Sync and Activation) in parallel with the first x chunk, so the first
  real matmul is gated by ~one small transfer + the fixed DGE/semaphore
  latency instead of the full 0.6MB weight DMA.
- Warmup matmuls run on a memset scratch tile (no DMA dependency) and are
  sized to bridge until the weights land, so the PE p-state ramp overlaps
  the DMA head and the first real matmuls run at full clock.
- The final block's bias add is split across DVE and Activation so the tail
  add is ~2x shorter. Output DMAs are issued from Sync (idle after the
  input issues).
- Custom tile epilogue: a single-engine (Sync) drain carrying the tile
  clock waits instead of the fused all-engine drain barrier; Vector and
  GpSimd gate on it via a semaphore before the semaphore clears, since
  their NEFF-postamble clear ranges cover the live kernel semaphores.
  (The NEFF postamble's own pre-clear barrier still serializes the final
  ~7us of per-engine semaphore-file clears; that part is fixed cost.)
"""

import numpy as np
from ml_dtypes import bfloat16

import concourse.bass as bass
import concourse.tile as tile
from concourse import bacc, mybir
from concourse.bass_utils import run_bass_kernel_spmd

N, C, F, H, W = 32, 128, 128, 32, 32
N_CORES = 8
N_LOC = N // N_CORES  # images per core
HP, WP = H + 2, W + 2  # host-padded image
HB = 16      # rows per PSUM block (16*32 = 512 f32 = one PSUM bank)
HC = HB + 2  # rows per x chunk (chunk hb covers padded rows 16*hb .. +18)
N_WARM = 7   # warmup matmuls on the memset scratch tile
TAPS_A = 3   # taps 0..2 (ky=0) in the first weight shard, 3..8 in the second

F32 = mybir.dt.float32
BF16 = mybir.dt.bfloat16


def _partial_drain_and_barrier(self, tick_clock, wait_clock):
    """Tile epilogue: drain, but gate only Vector+GpSimd on it.

    The NEFF postamble makes every engine serially zero its share of the
    256-semaphore file (Tensor S4..53, Scalar S54..104, GpSimd S105..155,
    Vector S156..206, Sync S207..255) before the final barrier; at ~50-115ns
    per clear this is the dominant tail cost. Kernel semaphores live in
    [151, ~170], i.e. only in the GpSimd+Vector ranges, and Sync's own
    postamble DRAIN already waits for its DGE queue (the output DMAs). So
    Tensor/Scalar/Sync can run their clears concurrently with the output
    DMA tail; only Vector and GpSimd must wait for the drain so they don't
    zero a semaphore that an in-flight DMA completion still increments.
    """
    from concourse.vector_clock import ScopedClock

    nc = self.nc
    drain_inst = nc.sync.drain(fusable=False)
    wait_clock.add_sem_waits(
        drain_inst.ins, ScopedClock({None: tick_clock.global_clock})
    )
    gate = nc.alloc_semaphore("post_drain_gate")
    drain_inst.then_inc(gate, 1)
    nc.vector.wait_ge(gate, 1)
    nc.gpsimd.wait_ge(gate, 1)
    popped = nc._tile_sem_poison_stack.pop()
    assert popped is self._sem_poison
    nc.clear_and_free_semaphores(list(self.sems.allocated().values()))


def _build_module():
    nc = bacc.Bacc(None, dynamic_dma_scratch_size=256)

    # x is stored as per-block 18-row chunks (rows 16*hb .. 16*hb+18 of the
    # padded image), so each (image, hb) PSUM block reads exactly one chunk.
    x_d = nc.dram_tensor(
        "x", [N_LOC, H // HB, C, HC, WP], BF16, kind="ExternalInput"
    )
    wa_d = nc.dram_tensor("wa", [C, TAPS_A * F], BF16, kind="ExternalInput")
    wb_d = nc.dram_tensor("wb", [C, (9 - TAPS_A) * F], BF16, kind="ExternalInput")
    b_d = nc.dram_tensor("b", [F, 1], F32, kind="ExternalInput")
    o_d = nc.dram_tensor("out", [N_LOC, F, H, W], F32, kind="ExternalOutput")

    n_blocks = N_LOC * (H // HB)

    tile.TileContext._drain_and_barrier = _partial_drain_and_barrier
    with tile.TileContext(nc) as tc:
        with (
            tc.tile_pool(name="const", bufs=1) as cpool,
            tc.tile_pool(name="x", bufs=n_blocks) as xpool,
            tc.tile_pool(name="o", bufs=n_blocks) as opool,
            tc.tile_pool(name="ps", bufs=8, space=bass.MemorySpace.PSUM) as ppool,
        ):
            # Warmup path: a memset scratch tile means the warmup matmuls
            # depend on no DMA at all, so the PE HAM clock ramp runs while
            # the weight/x DMAs are still in flight.
            warm_sb = cpool.tile([C, HB * W], BF16)
            nc.gpsimd.memset(warm_sb[:], 1.0)
            ps_warm = ppool.tile([F, HB, W], F32, tag="ps")
            prev_mm = None
            for _ in range(N_WARM):
                mm = nc.tensor.matmul(
                    ps_warm[:],
                    warm_sb[:, 0:F],
                    warm_sb[:],
                    start=True,
                    stop=True,
                )
                if prev_mm is not None:
                    tile.add_dep_helper(
                        mm.ins, prev_mm.ins, sync=False, reason="PE program order"
                    )
                prev_mm = mm

            # Input DMA issues split across the two hardware DGE queues:
            # Sync carries x chunk 0 + the remaining chunks; Activation
            # carries the two weight shards + bias in parallel.
            wa_sb = cpool.tile([C, TAPS_A * F], BF16)
            wb_sb = cpool.tile([C, (9 - TAPS_A) * F], BF16)
            b_sb = cpool.tile([F, 1], F32)

            x_sbs = {}
            def load_chunk(n, hb):
                x_sb = xpool.tile([C, HC, WP], BF16, tag="x")
                nc.sync.dma_start(x_sb[:], x_d[n, hb])
                x_sbs[(n, hb)] = x_sb

            nc.scalar.dma_start(wa_sb[:], wa_d[:])
            load_chunk(0, 0)
            nc.scalar.dma_start(wb_sb[:], wb_d[:])
            load_chunk(0, 1)
            nc.scalar.dma_start(b_sb[:], b_d[:])
            for n in range(1, N_LOC):
                for hb in range(H // HB):
                    load_chunk(n, hb)

            def tap_lhsT(t):
                if t < TAPS_A:
                    return wa_sb[:, t * F : (t + 1) * F]
                return wb_sb[:, (t - TAPS_A) * F : (t - TAPS_A + 1) * F]

            for n in range(N_LOC):
                for hb in range(H // HB):
                    blk = n * (H // HB) + hb
                    x_sb = x_sbs[(n, hb)]
                    ps = ppool.tile([F, HB, W], F32, tag="ps")
                    for i, (ky, kx) in enumerate(
                        [(ky, kx) for ky in range(3) for kx in range(3)]
                    ):
                        rhs = x_sb[:, ky : ky + HB, kx : kx + W]
                        mm = nc.tensor.matmul(
                            ps[:],
                            tap_lhsT(ky * 3 + kx),
                            rhs,
                            start=(i == 0),
                            stop=(i == 8),
                        )
                        if prev_mm is not None:
                            # keep PE issue order = program order
                            tile.add_dep_helper(
                                mm.ins, prev_mm.ins, sync=False,
                                reason="PE program order",
                            )
                        prev_mm = mm
                    # bias add PSUM -> SBUF, then store this block from the
                    # Sync queue (idle once the input issues are done).
                    o_sb = opool.tile([F, HB, W], F32, tag="o")
                    if blk == n_blocks - 1:
                        # tail block: single DVE add (the Activation engine
                        # observes the PE semaphore ~0.6us late, so a split
                        # across both engines is no faster than DVE alone)
                        nc.vector.tensor_scalar_add(o_sb[:], ps[:], b_sb[:, 0:1])
                    elif blk % 2 == 0:
                        nc.vector.tensor_scalar_add(o_sb[:], ps[:], b_sb[:, 0:1])
                    else:
                        nc.scalar.add(o_sb[:], ps[:], b_sb[:, 0:1])
                    nc.sync.dma_start(
                        o_d[n][:, hb * HB : hb * HB + HB, :], o_sb[:]
                    )
    nc.compile()
    return nc


_NC_CACHE = None


def _run(x, weight, bias, **kwargs):
    global _NC_CACHE
    if _NC_CACHE is None:
        _NC_CACHE = _build_module()
    nc = _NC_CACHE

    xp = np.zeros((N, C, HP, WP), dtype=bfloat16)
    xp[:, :, 1 : 1 + H, 1 : 1 + W] = np.asarray(x, dtype=np.float32).astype(bfloat16)
    # per-block 18-row chunks: chunk hb = padded rows 16*hb .. 16*hb+18
    xc = np.stack([xp[:, :, 0:HC, :], xp[:, :, HB : HB + HC, :]], axis=1)
    # lhsT layout: w_pack[c, (ky*3+kx)*F + f] = weight[f, c, ky, kx]
    w_pack = np.ascontiguousarray(
        np.asarray(weight, dtype=np.float32)
        .transpose(1, 2, 3, 0)
        .reshape(C, 9 * F)
        .astype(bfloat16)
    )
    wa = np.ascontiguousarray(w_pack[:, : TAPS_A * F])
    wb = np.ascontiguousarray(w_pack[:, TAPS_A * F :])
    b2 = np.ascontiguousarray(np.asarray(bias, dtype=np.float32).reshape(F, 1))

    shards = xc.reshape(N_CORES, N_LOC, H // HB, C, HC, WP)
    in_maps = [
        {"x": shards[i], "wa": wa, "wb": wb, "b": b2} for i in range(N_CORES)
    ]
    return run_bass_kernel_spmd(nc, in_maps, core_ids=list(range(N_CORES)), **kwargs)


def kernel(x: np.ndarray, weight: np.ndarray, bias: np.ndarray, **_) -> np.ndarray:
    res = _run(x, weight, bias)
    return np.concatenate([res.results[i]["out"] for i in range(N_CORES)], axis=0)


# revision 12
# speedup vs baseline: 1.0454x; 1.0454x over previous
"""Conv2dfft kernel for Trainium2 (8 NeuronCores, SPMD data-parallel over N).

The reference computes an FFT-based 2D cross-correlation that is exactly a
3x3 same-padding conv2d: out[n,f,h,w] = sum_{c,ky,kx} x[n,c,h+ky-1,w+kx-1]
* weight[f,c,ky,kx] + bias[f]  (zero-padded at the borders).

We implement it directly as 9 shifted 128x128 matmuls accumulated in PSUM:
the contraction dim C=128 fills the PE partition dim, F=128 fills the output
partition dim. Data-parallel: 32 images / 8 cores = 4 images per core.

Trace-driven layout:
- fp32r and bf16 stream the PE at the same 1 row/cycle steady rate, so the
  matmul dtype does not change compute time -- but bf16 halves the input DMA
  bytes, which is what gates the FIRST matmul. x and w are shipped as bf16.
- The weight tensor is split in two and issued on BOTH hardware DGE queues
  (Sync and Activation) in parallel with the first x chunk, so the first
  real matmul is gated by ~one small transfer + the fixed DGE/semaphore
  latency instead of the full 0.6MB weight DMA.
- Warmup matmuls run on a memset scratch tile (no DMA dependency) and are
  sized to bridge until the weights land, so the PE p-state ramp overlaps
  the DMA head and the first real matmuls run at full clock.
- The final block's bias add is split across DVE and Activation so the tail
  add is ~2x shorter. Output DMAs are issued from Sync (idle after the
  input issues).
- Custom tile epilogue: a single-engine (Sync) drain carrying the tile
  clock waits instead of the fused all-engine drain barrier; Vector and
  GpSimd gate on it via a semaphore before the semaphore clears, since
  their NEFF-postamble clear ranges cover the live kernel semaphores.
  (The NEFF postamble's own pre-clear barrier still serializes the final
  ~7us of per-engine semaphore-file clears; that part is fixed cost.)
"""

import numpy as np
from ml_dtypes import bfloat16

import concourse.bass as bass
import concourse.tile as tile
from concourse import bacc, mybir
from concourse.bass_utils import run_bass_kernel_spmd

N, C, F, H, W = 32, 128, 128, 32, 32
N_CORES = 8
N_LOC = N // N_CORES  # images per core
HP, WP = H + 2, W + 2  # host-padded image
HB = 16      # rows per PSUM block (16*32 = 512 f32 = one PSUM bank)
HC = HB + 2  # rows per x chunk (chunk hb covers padded rows 16*hb .. +18)
N_WARM = 7   # warmup matmuls on the memset scratch tile
TAPS_A = 5   # taps 0..4 in the first weight shard, 5..8 in the second

F32 = mybir.dt.float32
BF16 = mybir.dt.bfloat16


def _partial_drain_and_barrier(self, tick_clock, wait_clock):
    """Tile epilogue: drain, but gate only Vector+GpSimd on it.

    The NEFF postamble makes every engine serially zero its share of the
    256-semaphore file (Tensor S4..53, Scalar S54..104, GpSimd S105..155,
    Vector S156..206, Sync S207..255) before the final barrier; at ~50-115ns
    per clear this is the dominant tail cost. Kernel semaphores live in
    [151, ~170], i.e. only in the GpSimd+Vector ranges, and Sync's own
    postamble DRAIN already waits for its DGE queue (the output DMAs). So
    Tensor/Scalar/Sync can run their clears concurrently with the output
    DMA tail; only Vector and GpSimd must wait for the drain so they don't
    zero a semaphore that an in-flight DMA completion still increments.
    """
    from concourse.vector_clock import ScopedClock

    nc = self.nc
    drain_inst = nc.sync.drain(fusable=False)
    wait_clock.add_sem_waits(
        drain_inst.ins, ScopedClock({None: tick_clock.global_clock})
    )
    gate = nc.alloc_semaphore("post_drain_gate")
    drain_inst.then_inc(gate, 1)
    nc.vector.wait_ge(gate, 1)
    nc.gpsimd.wait_ge(gate, 1)
    popped = nc._tile_sem_poison_stack.pop()
    assert popped is self._sem_poison
    nc.clear_and_free_semaphores(list(self.sems.allocated().values()))


def _build_module():
    nc = bacc.Bacc(None, dynamic_dma_scratch_size=256)

    # x is stored as per-block 18-row chunks (rows 16*hb .. 16*hb+18 of the
    # padded image), so each (image, hb) PSUM block reads exactly one chunk.
    x_d = nc.dram_tensor(
        "x", [N_LOC, H // HB, C, HC, WP], BF16, kind="ExternalInput"
    )
    wa_d = nc.dram_tensor("wa", [C, TAPS_A * F], BF16, kind="ExternalInput")
    wb_d = nc.dram_tensor("wb", [C, (9 - TAPS_A) * F], BF16, kind="ExternalInput")
    b_d = nc.dram_tensor("b", [F, 1], F32, kind="ExternalInput")
    o_d = nc.dram_tensor("out", [N_LOC, F, H, W], F32, kind="ExternalOutput")

    n_blocks = N_LOC * (H // HB)

    tile.TileContext._drain_and_barrier = _partial_drain_and_barrier
    with tile.TileContext(nc) as tc:
        with (
            tc.tile_pool(name="const", bufs=1) as cpool,
            tc.tile_pool(name="x", bufs=n_blocks) as xpool,
            tc.tile_pool(name="o", bufs=n_blocks) as opool,
            tc.tile_pool(name="ps", bufs=8, space=bass.MemorySpace.PSUM) as ppool,
        ):
            # Warmup path: a memset scratch tile means the warmup matmuls
            # depend on no DMA at all, so the PE HAM clock ramp runs while
            # the weight/x DMAs are still in flight.
            warm_sb = cpool.tile([C, HB * W], BF16)
            nc.gpsimd.memset(warm_sb[:], 1.0)
            ps_warm = ppool.tile([F, HB, W], F32, tag="ps")
            prev_mm = None
            for _ in range(N_WARM):
                mm = nc.tensor.matmul(
                    ps_warm[:],
                    warm_sb[:, 0:F],
                    warm_sb[:],
                    start=True,
                    stop=True,
                )
                if prev_mm is not None:
                    tile.add_dep_helper(
                        mm.ins, prev_mm.ins, sync=False, reason="PE program order"
                    )
                prev_mm = mm

            # Input DMA issues split across the two hardware DGE queues:
            # Sync carries x chunk 0 + the remaining chunks; Activation
            # carries the two weight shards + bias in parallel.
            wa_sb = cpool.tile([C, TAPS_A * F], BF16)
            wb_sb = cpool.tile([C, (9 - TAPS_A) * F], BF16)
            b_sb = cpool.tile([F, 1], F32)

            x_sbs = {}
            def load_chunk(n, hb):
                x_sb = xpool.tile([C, HC, WP], BF16, tag="x")
                nc.sync.dma_start(x_sb[:], x_d[n, hb])
                x_sbs[(n, hb)] = x_sb

            nc.scalar.dma_start(wa_sb[:], wa_d[:])
            load_chunk(0, 0)
            nc.scalar.dma_start(wb_sb[:], wb_d[:])
            load_chunk(0, 1)
            nc.scalar.dma_start(b_sb[:], b_d[:])
            for n in range(1, N_LOC):
                for hb in range(H // HB):
                    load_chunk(n, hb)

            def tap_lhsT(t):
                if t < TAPS_A:
                    return wa_sb[:, t * F : (t + 1) * F]
                return wb_sb[:, (t - TAPS_A) * F : (t - TAPS_A + 1) * F]

            for n in range(N_LOC):
                for hb in range(H // HB):
                    blk = n * (H // HB) + hb
                    x_sb = x_sbs[(n, hb)]
                    ps = ppool.tile([F, HB, W], F32, tag="ps")
                    for i, (ky, kx) in enumerate(
                        [(ky, kx) for ky in range(3) for kx in range(3)]
                    ):
                        rhs = x_sb[:, ky : ky + HB, kx : kx + W]
                        mm = nc.tensor.matmul(
                            ps[:],
                            tap_lhsT(ky * 3 + kx),
                            rhs,
                            start=(i == 0),
                            stop=(i == 8),
                        )
                        if prev_mm is not None:
                            # keep PE issue order = program order
                            tile.add_dep_helper(
                                mm.ins, prev_mm.ins, sync=False,
                                reason="PE program order",
                            )
                        prev_mm = mm
                    # bias add PSUM -> SBUF, then store this block from the
                    # Sync queue (idle once the input issues are done).
                    o_sb = opool.tile([F, HB, W], F32, tag="o")
                    if blk == n_blocks - 1:
                        # tail block: split the add across both engines
                        nc.vector.tensor_scalar_add(
                            o_sb[:, 0 : HB // 2, :], ps[:, 0 : HB // 2, :],
                            b_sb[:, 0:1],
                        )
                        nc.scalar.add(
                            o_sb[:, HB // 2 : HB, :], ps[:, HB // 2 : HB, :],
                            b_sb[:, 0:1],
                        )
                    elif blk % 2 == 0:
                        nc.vector.tensor_scalar_add(o_sb[:], ps[:], b_sb[:, 0:1])
                    else:
                        nc.scalar.add(o_sb[:], ps[:], b_sb[:, 0:1])
                    nc.sync.dma_start(
                        o_d[n][:, hb * HB : hb * HB + HB, :], o_sb[:]
                    )
    nc.compile()
    return nc


_NC_CACHE = None


def _run(x, weight, bias, **kwargs):
    global _NC_CACHE
    if _NC_CACHE is None:
        _NC_CACHE = _build_module()
    nc = _NC_CACHE

    xp = np.zeros((N, C, HP, WP), dtype=bfloat16)
    xp[:, :, 1 : 1 + H, 1 : 1 + W] = np.asarray(x, dtype=np.float32).astype(bfloat16)
    # per-block 18-row chunks: chunk hb = padded rows 16*hb .. 16*hb+18
    xc = np.stack([xp[:, :, 0:HC, :], xp[:, :, HB : HB + HC, :]], axis=1)
    # lhsT layout: w_pack[c, (ky*3+kx)*F + f] = weight[f, c, ky, kx]
    w_pack = np.ascontiguousarray(
        np.asarray(weight, dtype=np.float32)
        .transpose(1, 2, 3, 0)
        .reshape(C, 9 * F)
        .astype(bfloat16)
    )
    wa = np.ascontiguousarray(w_pack[:, : TAPS_A * F])
    wb = np.ascontiguousarray(w_pack[:, TAPS_A * F :])
    b2 = np.ascontiguousarray(np.asarray(bias, dtype=np.float32).reshape(F, 1))

    shards = xc.reshape(N_CORES, N_LOC, H // HB, C, HC, WP)
    in_maps = [
        {"x": shards[i], "wa": wa, "wb": wb, "b": b2} for i in range(N_CORES)
    ]
    return run_bass_kernel_spmd(nc, in_maps, core_ids=list(range(N_CORES)), **kwargs)


def kernel(x: np.ndarray, weight: np.ndarray, bias: np.ndarray, **_) -> np.ndarray:
    res = _run(x, weight, bias)
    return np.concatenate([res.results[i]["out"] for i in range(N_CORES)], axis=0)
